# revision 8
# baseline (speedup 1.0000x reference)
"""KNRM forward on 8 Trainium2 NeuronCores via a hand-written Bass/Tile kernel.

Sharding: data-parallel over batch B=4096 -> 512 items/core; the (host-
pre-normalized) embedding table is replicated on every core.

Per-core device pipeline (per predict, 16 super-groups (SG) of 32 items):
  1. indirect-DMA gather of q (1024 rows) and d (4096 rows) embedding rows
     -> SBUF [token, D] tiles
  2. PE transpose -> [D, token] operand slabs (eqT, edT)
  3. per-item fp32 matmul M = eqT.T @ edT -> PSUM M-slab [128(4 items x 32 lq), 1024(8 groups x 128 ld)]
  4. 21 RBF kernel pools over M:
       - z = exp(10M), zi = exp(-10M), u0 = exp(-50(M-.05)^2) (ACT)
       - middle kernels via multiplicative chain u_{k+1} = u_k * z with
         fused reduce (DVE tensor_tensor_reduce)
       - outer kernels direct: Square(M-mu) + Exp(accum) (ACT)
       - exact-match kernel (sigma=1e-3) via is_ge(M, thr) with fused accum
  5. end phase: log1p (ACT Ln w/ per-kernel scale), sum over lq via PE
     matmul with block-ones, MLP 21->10->5->1 via block-diagonal weights on
     PE, sigmoid(l1 - l2) via Exp + reciprocal, DMA out.

Host keeps a cached jitted PJRT dispatch (axon) with the emb table resident
on-device; per-call transfers are only the int32 index tensors + [4096,1] out.
"""

import os
import sys

import numpy as np

sys.path.insert(0, "/opt/trn_rl_repo")

# ---------------------------------------------------------------- constants
K_NUM = 21
SIGMA = 0.1
EXACT_SIGMA = 0.001
_step = 1.0 / (K_NUM - 1)
_ar1 = np.linspace(_step, 1 - _step, (K_NUM - 1) // 2, endpoint=True)
MUS = np.hstack((-_ar1[::-1], _ar1, np.array([1.0]))).astype(np.float64)  # [21]

VOCAB, D = 100000, 128
B, LQ, LD = 4096, 32, 128
N_CORES = 8
BC = B // N_CORES          # items per core (512)
P = 128                    # partitions

GROUP = 4                  # items per partition-group (4 x 32 lq = 128)
SG_ITEMS = 32              # items per super-group
SG_GROUPS = SG_ITEMS // GROUP   # 8 groups / SG
SG_FREE = SG_GROUPS * LD        # 1024 M columns / SG

# kernel split: chain covers k in [CHAIN_LO, CHAIN_HI] (mu=-0.95+0.1k),
# base of the chain is k=10 (mu=0.05); the rest are computed directly.
BASE_K = 10
CHAIN_LO = 4
CHAIN_HI = 17
DIRECT_KS = [k for k in range(20) if not (CHAIN_LO <= k <= CHAIN_HI)]
EXACT_K = 20
EXACT_THR = 0.997

# per-kernel scale folded into log1p:  feats_k = sum_l log(1 + scale_k * S_k)
# chain kernels store u_k = exp(-50(M-0.05)^2) * z^(k-10)
#   -> true kernel = u_k * exp(-50 mu_k^2 + 0.125)
def _k_scale(k: int) -> float:
    if k == EXACT_K:
        return 1.0
    if CHAIN_LO <= k <= CHAIN_HI:
        return float(np.exp(-50.0 * MUS[k] ** 2 + 0.125))
    return 1.0  # direct kernels computed exactly


# ---------------------------------------------------------------- bass build
def build_nc(bc: int = BC):
    """Build the per-core Bass program. bc = items per core (mult of 32)."""
    import concourse.bacc as bacc
    import concourse.bass as bass
    import concourse.tile as tile
    from concourse import mybir
    from concourse.masks import make_identity

    assert bc % SG_ITEMS == 0
    n_sg = bc // SG_ITEMS            # SGs per predict
    ng = bc // GROUP                 # groups per predict (S columns / kernel)
    nq_tiles = bc * LQ // P          # q gather tiles per predict
    nd_tiles = bc * LD // P          # d gather tiles per predict

    f32 = mybir.dt.float32
    i32 = mybir.dt.int32
    AF = mybir.ActivationFunctionType
    OP = mybir.AluOpType

    nc = bacc.Bacc(None, target_bir_lowering=False, debug=False)

    # ---- DRAM I/O
    embn = nc.dram_tensor("embn", [VOCAB, D], f32, kind="ExternalInput")
    qi = [nc.dram_tensor(f"qi{p}", [nq_tiles, P], i32, kind="ExternalInput")
          for p in (1, 2)]
    di = [nc.dram_tensor(f"di{p}", [nd_tiles, P], i32, kind="ExternalInput")
          for p in (1, 2)]
    w1e_d = nc.dram_tensor("w1e", [84, 40], f32, kind="ExternalInput")
    w2e_d = nc.dram_tensor("w2e", [40, 20], f32, kind="ExternalInput")
    w3e_d = nc.dram_tensor("w3e", [20, 4], f32, kind="ExternalInput")
    b1e_d = nc.dram_tensor("b1e", [40, 1], f32, kind="ExternalInput")
    b2e_d = nc.dram_tensor("b2e", [20, 1], f32, kind="ExternalInput")
    b3e_d = nc.dram_tensor("b3e", [4, 1], f32, kind="ExternalInput")
    out_d = nc.dram_tensor("out", [bc, 1], f32, kind="ExternalOutput")

    blk = np.zeros((P, GROUP), np.float32)
    for p in range(P):
        blk[p, p // LQ] = 1.0
    blkones_d = nc.inline_tensor(blk, name="blkones")
    mub = np.tile((-MUS[:20]).astype(np.float32), (P, 1))
    mubias_d = nc.inline_tensor(mub, name="mubias")

    with tile.TileContext(nc) as tc:
        cpool = tc.alloc_tile_pool(name="const", bufs=1)
        spool = tc.alloc_tile_pool(name="sbuf", bufs=2)
        upool = tc.alloc_tile_pool(name="chain", bufs=4)
        slabp = tc.alloc_tile_pool(name="slabs", bufs=2)
        ppool = tc.alloc_tile_pool(name="psum", bufs=2, space="PSUM")
        epool = tc.alloc_tile_pool(name="endpsum", bufs=2, space="PSUM")

        # ---- constants / prologue
        ident = cpool.tile([P, P], f32, tag="ident")
        make_identity(nc, ident[:])
        blkones = cpool.tile([P, GROUP], f32, tag="blkones")
        nc.sync.dma_start(out=blkones[:], in_=blkones_d[:])
        mubias = cpool.tile([P, 20], f32, tag="mubias")
        nc.sync.dma_start(out=mubias[:], in_=mubias_d[:])
        w1e = cpool.tile([84, 40], f32, tag="w1e")
        nc.sync.dma_start(out=w1e[:], in_=w1e_d[:])
        w2e = cpool.tile([40, 20], f32, tag="w2e")
        nc.sync.dma_start(out=w2e[:], in_=w2e_d[:])
        w3e = cpool.tile([20, 4], f32, tag="w3e")
        nc.sync.dma_start(out=w3e[:], in_=w3e_d[:])
        b1e = cpool.tile([40, 1], f32, tag="b1e")
        nc.sync.dma_start(out=b1e[:], in_=b1e_d[:])
        b2e = cpool.tile([20, 1], f32, tag="b2e")
        nc.sync.dma_start(out=b2e[:], in_=b2e_d[:])
        b3e = cpool.tile([4, 1], f32, tag="b3e")
        nc.sync.dma_start(out=b3e[:], in_=b3e_d[:])

        qidx = []
        didx = []
        for pr in range(2):
            qt = cpool.tile([P, nq_tiles], i32, tag=f"qidx{pr}")
            nc.sync.dma_start(out=qt[:], in_=qi[pr][:].rearrange("t p -> p t"))
            qidx.append(qt)
            dt_ = cpool.tile([P, nd_tiles], i32, tag=f"didx{pr}")
            nc.sync.dma_start(out=dt_[:], in_=di[pr][:].rearrange("t p -> p t"))
            didx.append(dt_)

        logits_sb = []

        for pr in range(2):
            S = slabp.tile([P, K_NUM * ng], f32, tag="S")

            for sg in range(n_sg):
                # ---------------- gather
                raw_q = spool.tile([P, 8 * D], f32, tag="rawq")
                nc.gpsimd.indirect_dma_start(
                    out=raw_q[:], out_offset=None,
                    in_=embn[:],
                    in_offset=bass.IndirectOffsetOnAxis(
                        ap=qidx[pr][:, sg * 8:(sg + 1) * 8], axis=0),
                )
                raw_d = spool.tile([P, 32 * D], f32, tag="rawd")
                nc.gpsimd.indirect_dma_start(
                    out=raw_d[:], out_offset=None,
                    in_=embn[:],
                    in_offset=bass.IndirectOffsetOnAxis(
                        ap=didx[pr][:, sg * 32:(sg + 1) * 32], axis=0),
                )

                # ---------------- transpose to [D, token]
                eqT = spool.tile([P, 8 * D], f32, tag="eqT")
                edT = spool.tile([P, 32 * D], f32, tag="edT")
                for half, (raw, dstT, ntile) in enumerate(
                        [(raw_q, eqT, 8), (raw_d, edT, 32)]):
                    for t0 in range(0, ntile, 4):
                        tsl = ppool.tile([P, 4 * D], f32, tag="tslab")
                        for t in range(t0, min(t0 + 4, ntile)):
                            nc.tensor.transpose(
                                out=tsl[:, (t - t0) * D:(t - t0 + 1) * D],
                                in_=raw[:, t * D:(t + 1) * D],
                                identity=ident[:],
                            )
                        nc.any.tensor_copy(
                            out=dstT[:, t0 * D:(t0 + 4) * D], in_=tsl[:])

                # ---------------- M matmuls (fp32, col-tiled 4 items/group)
                M = ppool.tile([P, SG_FREE], f32, tag="M")
                for i in range(SG_ITEMS):
                    g, j = i // GROUP, i % GROUP
                    nc.tensor.matmul(
                        out=M[32 * j:32 * (j + 1), g * LD:(g + 1) * LD],
                        lhsT=eqT[:, i * LQ:(i + 1) * LQ],
                        rhs=edT[:, i * LD:(i + 1) * LD],
                        start=True, stop=True,
                        tile_position=(0, 32 * j),
                    )

                gcol = sg * SG_GROUPS  # global group offset for S columns

                # ---------------- prep (ACT reads M from PSUM)
                z = upool.tile([P, SG_FREE], f32, tag="z")
                nc.scalar.activation(z[:], M[:], AF.Exp, scale=10.0)
                zi = upool.tile([P, SG_FREE], f32, tag="zi")
                nc.scalar.activation(zi[:], M[:], AF.Exp, scale=-10.0)
                sq = upool.tile([P, SG_FREE], f32, tag="sq")
                nc.scalar.activation(sq[:], M[:], AF.Square, bias=mubias[:, BASE_K:BASE_K + 1])
                u0 = upool.tile([P, SG_FREE], f32, tag="u")
                for g in range(SG_GROUPS):
                    nc.scalar.activation(
                        u0[:, g * LD:(g + 1) * LD], sq[:, g * LD:(g + 1) * LD],
                        AF.Exp, scale=-50.0,
                        accum_out=S[:, BASE_K * ng + gcol + g: BASE_K * ng + gcol + g + 1],
                    )

                # ---------------- exact-match kernel (k=20): count M >= thr
                junk = upool.tile([P, SG_FREE], f32, tag="junk")
                for g in range(SG_GROUPS):
                    nc.vector.tensor_scalar(
                        out=junk[:, g * LD:(g + 1) * LD],
                        in0=M[:, g * LD:(g + 1) * LD],
                        scalar1=EXACT_THR, scalar2=None, op0=OP.is_ge,
                        op1=OP.add,
                        accum_out=S[:, EXACT_K * ng + gcol + g: EXACT_K * ng + gcol + g + 1],
                    )

                # ---------------- chain kernels (fused mul+reduce)
                for direction, zfac, hi in ((1, z, CHAIN_HI), (-1, zi, CHAIN_LO)):
                    ucur = u0
                    k = BASE_K
                    while k != hi:
                        k += direction
                        unext = upool.tile([P, SG_FREE], f32, tag="u")
                        for g in range(SG_GROUPS):
                            nc.vector.tensor_tensor_reduce(
                                out=unext[:, g * LD:(g + 1) * LD],
                                in0=ucur[:, g * LD:(g + 1) * LD],
                                in1=zfac[:, g * LD:(g + 1) * LD],
                                scale=1.0, scalar=0.0,
                                op0=OP.mult, op1=OP.add,
                                accum_out=S[:, k * ng + gcol + g: k * ng + gcol + g + 1],
                            )
                        ucur = unext

                # ---------------- direct kernels
                for k in DIRECT_KS:
                    sqk = upool.tile([P, SG_FREE], f32, tag="sq")
                    nc.scalar.activation(sqk[:], M[:], AF.Square, bias=mubias[:, k:k + 1])
                    tk = upool.tile([P, SG_FREE], f32, tag="junk")
                    for g in range(SG_GROUPS):
                        nc.scalar.activation(
                            tk[:, g * LD:(g + 1) * LD], sqk[:, g * LD:(g + 1) * LD],
                            AF.Exp, scale=-50.0,
                            accum_out=S[:, k * ng + gcol + g: k * ng + gcol + g + 1],
                        )

            # ---------------- end phase for this predict
            feats_ps = epool.tile([ng, K_NUM * GROUP], f32, tag="endps")
            for k in range(K_NUM):
                logS = spool.tile([P, ng], f32, tag="logS", )
                nc.scalar.activation(
                    logS[:], S[:, k * ng:(k + 1) * ng], AF.Ln,
                    bias=1.0, scale=_k_scale(k),
                )
                nc.tensor.matmul(
                    out=feats_ps[:, k * GROUP:(k + 1) * GROUP],
                    lhsT=logS[:], rhs=blkones[:],
                    start=True, stop=True,
                )
            feats_sb = spool.tile([ng, K_NUM * GROUP], f32, tag="feats_sb")
            nc.any.tensor_copy(out=feats_sb[:], in_=feats_ps[:])
            featsT_ps = epool.tile([K_NUM * GROUP, ng], f32, tag="endps")
            nc.tensor.transpose(
                out=featsT_ps[:], in_=feats_sb[:], identity=ident[:ng, :ng])
            featsT = spool.tile([K_NUM * GROUP, ng], f32, tag="featsT")
            nc.any.tensor_copy(out=featsT[:], in_=featsT_ps[:])

            h1_ps = epool.tile([40, ng], f32, tag="endps")
            nc.tensor.matmul(out=h1_ps[:], lhsT=w1e[:], rhs=featsT[:],
                             start=True, stop=True)
            h1 = spool.tile([40, ng], f32, tag="h1")
            nc.scalar.activation(h1[:], h1_ps[:], AF.Relu, bias=b1e[:])

            h2_ps = epool.tile([20, ng], f32, tag="endps")
            nc.tensor.matmul(out=h2_ps[:], lhsT=w2e[:], rhs=h1[:],
                             start=True, stop=True)
            h2 = spool.tile([20, ng], f32, tag="h2")
            nc.scalar.activation(h2[:], h2_ps[:], AF.Relu, bias=b2e[:])

            l_ps = epool.tile([GROUP, ng], f32, tag="endps")
            nc.tensor.matmul(out=l_ps[:], lhsT=w3e[:], rhs=h2[:],
                             start=True, stop=True)
            lg = slabp.tile([GROUP, ng], f32, tag="logits")
            nc.scalar.activation(lg[:], l_ps[:], AF.Identity, bias=b3e[:])
            logits_sb.append(lg)

        # ---------------- sigmoid(l1 - l2) and output
        diff = spool.tile([GROUP, ng], f32, tag="diff")
        nc.vector.tensor_tensor(
            out=diff[:], in0=logits_sb[0][:], in1=logits_sb[1][:],
            op=OP.subtract)
        ex = spool.tile([GROUP, ng], f32, tag="ex")
        nc.scalar.activation(ex[:], diff[:], AF.Exp, scale=-1.0)
        den = spool.tile([GROUP, ng], f32, tag="den")
        nc.vector.tensor_scalar(
            out=den[:], in0=ex[:], scalar1=1.0, scalar2=None, op0=OP.add)
        sig = spool.tile([GROUP, ng], f32, tag="sig")
        nc.vector.reciprocal(out=sig[:], in_=den[:])
        nc.sync.dma_start(
            out=out_d[:].rearrange("(g j) one -> j (g one)", j=GROUP),
            in_=sig[:])

        for pl in (epool, ppool, slabp, upool, spool, cpool):
            pl.release()

    nc.compile()
    return nc


# ---------------------------------------------------------------- host prep
_cache = {}


def _prep_weights(W1, b1, W2, b2, W3, b3):
    key = ("weights", id(W1), id(W2), id(W3))
    hit = _cache.get(key)
    if hit is not None:
        return hit
    W1 = np.asarray(W1, np.float32)
    W2 = np.asarray(W2, np.float32)
    W3 = np.asarray(W3, np.float32)
    b1 = np.asarray(b1, np.float32).reshape(-1)
    b2 = np.asarray(b2, np.float32).reshape(-1)
    b3 = np.asarray(b3, np.float32).reshape(-1)
    w1e = np.zeros((84, 40), np.float32)
    for k in range(21):
        for j in range(4):
            w1e[k * 4 + j, :] .reshape(10, 4)[:, j] = W1[k]
    w2e = np.zeros((40, 20), np.float32)
    for m in range(10):
        for j in range(4):
            w2e[m * 4 + j].reshape(5, 4)[:, j] = W2[m]
    w3e = np.zeros((20, 4), np.float32)
    for m in range(5):
        for j in range(4):
            w3e[m * 4 + j, j] = W3[m, 0]
    b1e = np.repeat(b1, 4).astype(np.float32).reshape(40, 1)
    b2e = np.repeat(b2, 4).astype(np.float32).reshape(20, 1)
    b3e = np.full((4, 1), b3[0], np.float32)
    out = (w1e, w2e, w3e, b1e, b2e, b3e)
    _cache[key] = out
    return out


def _prep_emb(emb):
    key = ("embn", id(emb))
    hit = _cache.get(key)
    if hit is not None:
        return hit
    emb = np.asarray(emb, np.float32)
    n = np.sqrt((emb.astype(np.float32) ** 2).sum(-1, dtype=np.float32))
    embn = emb / np.maximum(n, 1e-12)[:, None]
    embn = np.ascontiguousarray(embn, np.float32)
    _cache[key] = embn
    return embn


# ---------------------------------------------------------------- runner
class _Runner:
    def __init__(self):
        import jax
        from jax.sharding import Mesh, PartitionSpec, NamedSharding
        try:
            from jax.experimental.shard_map import shard_map
        except Exception:
            from jax.shard_map import shard_map  # newer jax
        from concourse import bass2jax, mybir
        from concourse.bass2jax import _bass_exec_p, install_neuronx_cc_hook

        self.jax = jax
        self.nc = build_nc(BC)
        install_neuronx_cc_hook()

        m = self.nc.m
        in_names, out_names, out_avals, zero_outs = [], [], [], []
        for alloc in m.functions[0].allocations:
            if not isinstance(alloc, mybir.MemoryLocationSet):
                continue
            name = alloc.memorylocations[0].name
            if alloc.kind == "ExternalInput":
                in_names.append(name)
            elif alloc.kind == "ExternalOutput":
                out_names.append(name)
                shape = tuple(alloc.tensor_shape)
                dtype = mybir.dt.np(alloc.dtype)
                out_avals.append(jax.core.ShapedArray(shape, dtype))
                zero_outs.append(np.zeros(shape, dtype))
        self.in_names = in_names
        self.out_names = out_names
        n_params = len(in_names)
        all_names = in_names + out_names
        nc = self.nc

        def _body(*args):
            outs = _bass_exec_p.bind(
                *args,
                out_avals=tuple(out_avals),
                in_names=tuple(all_names),
                out_names=tuple(out_names),
                lowering_input_output_aliases=(),
                sim_require_finite=True,
                sim_require_nnan=True,
                nc=nc,
            )
            return tuple(outs)

        devices = jax.devices()[:N_CORES]
        self.mesh = Mesh(np.asarray(devices), ("core",))
        self.P = PartitionSpec
        self.NamedSharding = NamedSharding

        # replicated inputs (same array on every core)
        repl = {"embn", "w1e", "w2e", "w3e", "b1e", "b2e", "b3e"}
        self.repl = repl
        in_specs = tuple(
            PartitionSpec() if n in repl else PartitionSpec("core")
            for n in in_names
        ) + (PartitionSpec("core"),) * len(out_names)
        out_specs = (PartitionSpec("core"),) * len(out_names)
        donate = tuple(range(n_params, n_params + len(out_names)))
        self.zero_outs = zero_outs

        self.fn = jax.jit(
            shard_map(_body, mesh=self.mesh, in_specs=in_specs,
                      out_specs=out_specs, check_rep=False),
            donate_argnums=donate, keep_unused=True)

    def put_repl(self, name, arr):
        key = ("dev", name, id(arr))
        hit = _cache.get(key)
        if hit is not None:
            return hit
        dev = self.jax.device_put(
            arr, self.NamedSharding(self.mesh, self.P()))
        dev.block_until_ready()
        _cache[key] = dev
        return dev

    def __call__(self, host_ins):
        args = []
        for n in self.in_names:
            a = host_ins[n]
            if n in self.repl:
                a = self.put_repl(n, a)
            args.append(a)
        for z in self.zero_outs:
            args.append(np.zeros((N_CORES * z.shape[0],) + z.shape[1:],
                                 z.dtype))
        outs = self.fn(*args)
        return np.asarray(outs[0])


_runner = None


def kernel(emb, query_1, doc_1, query_2, doc_2, W1, b1, W2, b2, W3, b3):
    global _runner
    if _runner is None:
        _runner = _Runner()
    embn = _prep_emb(emb)
    w1e, w2e, w3e, b1e, b2e, b3e = _prep_weights(W1, b1, W2, b2, W3, b3)
    host_ins = {
        "embn": embn,
        "qi1": np.ascontiguousarray(np.asarray(query_1, np.int32).reshape(-1, P)),
        "di1": np.ascontiguousarray(np.asarray(doc_1, np.int32).reshape(-1, P)),
        "qi2": np.ascontiguousarray(np.asarray(query_2, np.int32).reshape(-1, P)),
        "di2": np.ascontiguousarray(np.asarray(doc_2, np.int32).reshape(-1, P)),
        "w1e": w1e, "w2e": w2e, "w3e": w3e,
        "b1e": b1e, "b2e": b2e, "b3e": b3e,
    }
    out = _runner(host_ins)  # [4096, 1]
    return np.asarray(out, np.float32).reshape(B, 1)
